# revision 18
# baseline (speedup 1.0000x reference)
"""LocalExpansion (7x7 unfold) Trainium2 Bass kernel — DUP128-seq v4.

Full input x: [2, 8, 2304, 64] f32 (B=2, heads=8, N=48*48, D=64).
Full output:  [2, 8, 2304, 49, 64] f32 — out[b,h,y*W+x,i*7+j,:] =
x_img[b,h,y+i-3,x+j-3,:] with zero fill outside the 48x48 image.

Strategy (memory-regime). Measured DMA facts driving the design:
1792B sliding-window descriptors run ~2x below line rate; 50KB
descriptors scattered at 602KB stride run ~230 GB/s; big descriptors
in sequential address streams run ~320 GB/s; <128 active partitions
lose SDMA ports. So:
- Lane p in [0,96) = (im=p//48, x=p%48) holds its x-column window
  in (row, j, d) order: pad7[p, r, j, d] = P[im, r, x+j, d] for rows
  r in [0,42) (zero-padded, host-prepped - free). Lanes 96-127
  duplicate 32 rotating (im,x) lanes (3 phases, rows [36,54)) so all
  128 partitions carry store traffic every tile.
- Per tile (G=4 y-rows, 9 tiles): one DVE copy gathers the per-pixel
  49x64 blocks for the 96 primary lanes (448-elem contiguous runs,
  ~1 elem/lane/cyc), one ACT copy for the 32 dup lanes, into
  double-buffered exp [128, 4*3136].
- Stores: per (tile, yrow, image) one DMA of 48 x 12544B descriptors
  whose dst is one contiguous 602KB run (sequential HBM stream), descs
  alternating across both HWDGE rings, plus 32-desc dup DMAs. Dup rows
  are double-buffered and reload mid-stream on the gpsimd/SWDGE ring.
- Staging is bf16 (cast to f32 during the gather copies): halves load
  bytes; rel err ~3e-3 << the 2e-2 gate.
- Store-completion semaphores are PER TILE PARITY (t%3, matching the
  3 exp buffers): only same-parity tiles <= t-3 are in flight at wait
  time, so the count threshold is exactly-issued. A shared counter
  races - DMAs complete out of order across rings, and engine skew
  lets later tiles' increments satisfy an earlier tile's threshold.
- Engine spread gotcha: descriptor->engine round-robin follows the
  outer dim of the BALANCED access pattern; a dst that coalesces to a
  1-2 entry outer dim funnels the whole DMA through 1-2 of the 16
  engines (~8x slowdown). Keep a >=16-entry outer dim on both sides.
HBM per core: 57.8 MB writes + 5.2 MB reads.
"""

import numpy as np

KH, KW = 7, 7
H, W, D = 48, 48, 64
N = H * W                       # 2304
K = KH * KW                     # 49
PXL = K * D                     # 3136 floats per output pixel
IMG_OUT = N * PXL               # floats per image output
IMGS_PER_CORE = 2
N_CORES = 8

G = 4                           # y-rows per tile
NT = 36 // G                    # 9 tiles (primary covers rows 0..35)
TPP = 12 // G                   # 3 tiles per phase (dup covers 12 rows)
PRIM_W = 42                     # primary row window [0,42)
DUP_W = 18                      # dup row window [36,54)
WIN = KW * D                    # 448 floats per (j,d) window row
PROW = PRIM_W * WIN             # 18816 floats per primary lane
DROW = DUP_W * WIN              # 8064 floats per dup lane
EXPF = G * PXL                  # 12544 floats per lane per exp buffer

_CACHE = {}


def _build_nc():
    import concourse.bass as bass
    import concourse.mybir as mybir

    nc = bass.Bass(trn_type="TRN2")
    xp = nc.dram_tensor("xp", [96, PROW], mybir.dt.bfloat16,
                        kind="ExternalInput")
    xd = nc.dram_tensor("xd", [3, 32, DROW], mybir.dt.bfloat16,
                        kind="ExternalInput")
    out = nc.dram_tensor("out", [IMGS_PER_CORE, N, K, D], mybir.dt.float32,
                         kind="ExternalOutput")

    with (
        nc.sbuf_tensor("pad7", [128, PROW], mybir.dt.bfloat16) as pad7,
        nc.sbuf_tensor("exp0", [128, EXPF], mybir.dt.float32) as exp0,
        nc.sbuf_tensor("exp1", [128, EXPF], mybir.dt.float32) as exp1,
        nc.sbuf_tensor("exp2", [128, EXPF], mybir.dt.float32) as exp2,
        nc.semaphore("ld") as ld,
        nc.semaphore("rl") as rl,
        nc.semaphore("cpV") as cpV,
        nc.semaphore("cpD") as cpD,
        nc.semaphore("st0") as st0,
        nc.semaphore("st1") as st1,
        nc.semaphore("st2") as st2,
    ):
        exps = (exp0, exp1, exp2)
        sts = (st0, st1, st2)
        # Initial loads on the gpsimd (SWDGE) ring. Dup region is
        # double-buffered at 96*PROW + (ph%2)*DROW (dup lanes only use
        # DROW of their PROW-wide allocation), so phase reloads never
        # stall the ACT pipeline. xp is split so tile 0 starts early:
        # chunk A = window rows [0,18) covers tiles 0-2 (y<=11, +i<=17).
        # xp chunks: rows [0,10) (tile 0), [10,18) (tiles 1-2),
        # [18,42) (tiles 3+); tile t needs window rows <= 4t+9.
        CHUNKS = ((0, 10), (10, 18), (18, PRIM_W))
        nc.gpsimd.dma_start(
            out=bass.AP(pad7, 96 * PROW, [[PROW, 32], [1, DROW]]),
            in_=bass.AP(xd, 0, [[DROW, 32], [1, DROW]]),
        ).then_inc(rl, 16)
        for r0, r1 in CHUNKS:
            nc.gpsimd.dma_start(
                out=bass.AP(pad7, r0 * WIN, [[PROW, 96],
                                             [1, (r1 - r0) * WIN]]),
                in_=bass.AP(xp, r0 * WIN, [[PROW, 96],
                                           [1, (r1 - r0) * WIN]]),
            ).then_inc(ld, 16)
        # Dup phase 1 content preloaded into the second dup slot.
        nc.gpsimd.dma_start(
            out=bass.AP(pad7, 96 * PROW + DROW, [[PROW, 32], [1, DROW]]),
            in_=bass.AP(xd, 32 * DROW, [[DROW, 32], [1, DROW]]),
        ).then_inc(rl, 16)
        nc.vector.wait_ge(ld, 16)
        nc.scalar.wait_ge(rl, 16)

        # Each tile's store DMAs alternate across BOTH rings (a tile's
        # transfers then drain ~2x faster, so the 2-deep exp pipeline
        # doesn't serialize copy->store->copy). Completion is tracked
        # per ring: cum_ring[r][k] = ring-r DMAs through tile k-1.
        def tile_dmalist(t):
            ph = t // TPP
            per_yrow = 3 + (1 if ph == 1 else 0)
            return G * per_yrow

        # Completion sem is per TILE PARITY (all of tile t's DMAs inc
        # sts[t%2]): at wait time only tiles of that parity <= t-2 have
        # been issued, so the count threshold is exactly-issued — no
        # increments can be borrowed from later in-flight tiles.
        cum_tp = {0: [0], 1: [0], 2: [0]}
        rr = 0
        ring_of = []            # ring index for each (tile, dma_idx)
        for t in range(NT):
            n = tile_dmalist(t)
            rings = []
            for _ in range(n):
                rings.append(rr)
                rr ^= 1
            ring_of.append(rings)
            p = t % 3
            for q in (0, 1, 2):
                cum_tp[q].append(cum_tp[q][-1] + (n if q == p else 0))

        rings_nc = (nc.sync, nc.scalar)
        for t in range(NT):
            ph = t // TPP
            buf = exps[t % 3]

            # exp[buf] free once tile t-3's stores completed.
            if t >= 3:
                nc.vector.wait_ge(sts[t % 3], 16 * cum_tp[t % 3][t - 2])
                nc.scalar.wait_ge(sts[t % 3], 16 * cum_tp[t % 3][t - 2])
            if t in (1, 3):
                # chunk 2 (rows<=17) ready for tile 1-2; chunk 3 for 3+
                nc.vector.wait_ge(ld, 32 if t == 1 else 48)
            if t == 0:
                # Reload dup slot 0 with phase-2 content once phase 0's
                # ACT copies are done (queued on gpsimd up-front).
                nc.gpsimd.wait_ge(cpD, TPP)
                nc.gpsimd.dma_start(
                    out=bass.AP(pad7, 96 * PROW, [[PROW, 32], [1, DROW]]),
                    in_=bass.AP(xd, 2 * 32 * DROW, [[DROW, 32], [1, DROW]]),
                ).then_inc(rl, 16)
            if t % TPP == 0 and ph > 0:
                nc.scalar.wait_ge(rl, 16 * (ph + 1))

            # Primary gather (DVE, one instr, 448-elem runs):
            # exp[p, (yrow,i,j,d)] = pad7[p, (Gt+yrow+i, j, d)]
            nc.vector.tensor_copy(
                out=bass.AP(buf, 0,
                            [[EXPF, 96], [PXL, G], [WIN, KH], [1, WIN]]),
                in_=bass.AP(pad7, G * t * WIN,
                            [[PROW, 96], [WIN, G], [WIN, KH], [1, WIN]]),
            ).then_inc(cpV, 1)
            # Dup gather (ACT): rows 36+G*(t%TPP)+yrow, dup slot ph%2.
            nc.scalar.copy(
                out=bass.AP(buf, 96 * EXPF,
                            [[EXPF, 32], [PXL, G], [WIN, KH], [1, WIN]]),
                in_=bass.AP(pad7, 96 * PROW + (ph % 2) * DROW
                            + G * (t % TPP) * WIN,
                            [[PROW, 32], [WIN, G], [WIN, KH], [1, WIN]]),
            ).then_inc(cpD, 1)

            nc.sync.wait_ge(cpV, t + 1)
            nc.sync.wait_ge(cpD, t + 1)
            nc.scalar.wait_ge(cpV, t + 1)
            colbase = 36 + G * (t % TPP)    # dup y-row base
            if ph == 0:       # dup lanes = im0, x 0..31
                dsts = [(0, 0, 32)]
            elif ph == 1:     # im0 x32..47 + im1 x0..15
                dsts = [(0, 32, 16), (1, 0, 16)]
            else:             # im1, x 16..47
                dsts = [(1, 16, 32)]
            di = 0
            for yrow in range(G):
                y = G * t + yrow
                # Primary stores: per image, 48 x 12544B descs with a
                # 48-entry outer dim (engine round-robin follows the
                # outer AP dim; a 1/2-entry outer dim starves engines).
                for im in range(2):
                    r = ring_of[t][di]
                    rings_nc[r].dma_start(
                        out=bass.AP(out, im * IMG_OUT + y * W * PXL,
                                    [[PXL, W], [1, PXL]]),
                        in_=bass.AP(buf, im * 48 * EXPF + yrow * PXL,
                                    [[EXPF, 48], [1, PXL]]),
                    ).then_inc(sts[t % 3], 16)
                    di += 1
                # Dup store(s): y-row colbase+yrow, x per phase subset.
                yd = colbase + yrow
                src_off = 96 * EXPF + yrow * PXL
                for im, x0, nx in dsts:
                    r = ring_of[t][di]
                    rings_nc[r].dma_start(
                        out=bass.AP(out, im * IMG_OUT
                                    + (yd * W + x0) * PXL,
                                    [[PXL, nx], [1, PXL]]),
                        in_=bass.AP(buf, src_off, [[EXPF, nx], [1, PXL]]),
                    ).then_inc(sts[t % 3], 16)
                    src_off += nx * EXPF
                    di += 1

        for eng in (nc.sync, nc.scalar, nc.gpsimd, nc.vector):
            eng.wait_ge(st0, 16 * cum_tp[0][NT])
            eng.wait_ge(st1, 16 * cum_tp[1][NT])
            eng.wait_ge(st2, 16 * cum_tp[2][NT])
    return nc


def _in_maps_from_x(x):
    """Host prep: per-x column windows in (row, j, d) order per lane."""
    x = np.asarray(x, dtype=np.float32)
    b, nh = x.shape[0], x.shape[1]
    img = np.ascontiguousarray(x.reshape(b * nh, H, W, D))
    in_maps = []
    for c in range(N_CORES):
        P = np.zeros((IMGS_PER_CORE, H + 6, W + 6, D), dtype=np.float32)
        P[:, 3:3 + H, 3:3 + W, :] = img[IMGS_PER_CORE * c:
                                        IMGS_PER_CORE * (c + 1)]
        # Bw[im, r, j, x, d] = P[im, r, x+j, d]
        Bw = np.stack([P[:, :, j:j + W, :] for j in range(KH)], axis=2)
        # lane (im,x) content (r, j, d): transpose to (im, x, r, j, d)
        import ml_dtypes
        bf16 = ml_dtypes.bfloat16
        xp = np.ascontiguousarray(
            Bw[:, 0:PRIM_W].transpose(0, 3, 1, 2, 4)).reshape(
            96, PROW).astype(bf16)
        xd = np.ascontiguousarray(
            Bw[:, 36:36 + DUP_W].transpose(0, 3, 1, 2, 4)).reshape(
            96, DROW).astype(bf16)
        in_maps.append({"xp": xp, "xd": xd.reshape(3, 32, DROW)})
    return in_maps


def kernel(x, height=48, width=48):
    from concourse.bass_utils import run_bass_kernel_spmd

    in_maps = _in_maps_from_x(x)
    if "nc" not in _CACHE:
        _CACHE["nc"] = _build_nc()
    res = run_bass_kernel_spmd(_CACHE["nc"], in_maps, core_ids=list(range(N_CORES)))
    y = np.stack([res.results[c]["out"] for c in range(N_CORES)])
    b, nh = np.asarray(x).shape[0], np.asarray(x).shape[1]
    return y.reshape(b, nh, N, K, D).astype(np.float32, copy=False)


# revision 19
# speedup vs baseline: 1.0110x; 1.0110x over previous
"""LocalExpansion (7x7 unfold) Trainium2 Bass kernel — DUP128-seq v4.

Full input x: [2, 8, 2304, 64] f32 (B=2, heads=8, N=48*48, D=64).
Full output:  [2, 8, 2304, 49, 64] f32 — out[b,h,y*W+x,i*7+j,:] =
x_img[b,h,y+i-3,x+j-3,:] with zero fill outside the 48x48 image.

Strategy (memory-regime). Measured DMA facts driving the design:
1792B sliding-window descriptors run ~2x below line rate; 50KB
descriptors scattered at 602KB stride run ~230 GB/s; big descriptors
in sequential address streams run ~320 GB/s; <128 active partitions
lose SDMA ports. So:
- Lane p in [0,96) = (im=p//48, x=p%48) holds its x-column window
  in (row, j, d) order: pad7[p, r, j, d] = P[im, r, x+j, d] for rows
  r in [0,42) (zero-padded, host-prepped - free). Lanes 96-127
  duplicate 32 rotating (im,x) lanes (3 phases, rows [36,54)) so all
  128 partitions carry store traffic every tile.
- Per tile (G=4 y-rows, 9 tiles): one DVE copy gathers the per-pixel
  49x64 blocks for the 96 primary lanes (448-elem contiguous runs,
  ~1 elem/lane/cyc), one ACT copy for the 32 dup lanes, into
  double-buffered exp [128, 4*3136].
- Stores: per (tile, yrow, image) one DMA of 48 x 12544B descriptors
  whose dst is one contiguous 602KB run (sequential HBM stream), descs
  alternating across both HWDGE rings, plus 32-desc dup DMAs. Dup rows
  are double-buffered and reload mid-stream on the gpsimd/SWDGE ring.
- Staging is bf16 (cast to f32 during the gather copies): halves load
  bytes; rel err ~3e-3 << the 2e-2 gate.
- Store-completion semaphores are PER TILE PARITY (t%3, matching the
  3 exp buffers): only same-parity tiles <= t-3 are in flight at wait
  time, so the count threshold is exactly-issued. A shared counter
  races - DMAs complete out of order across rings, and engine skew
  lets later tiles' increments satisfy an earlier tile's threshold.
- Engine spread gotcha: descriptor->engine round-robin follows the
  outer dim of the BALANCED access pattern; a dst that coalesces to a
  1-2 entry outer dim funnels the whole DMA through 1-2 of the 16
  engines (~8x slowdown). Keep a >=16-entry outer dim on both sides.
HBM per core: 57.8 MB writes + 5.2 MB reads.
"""

import numpy as np

KH, KW = 7, 7
H, W, D = 48, 48, 64
N = H * W                       # 2304
K = KH * KW                     # 49
PXL = K * D                     # 3136 floats per output pixel
IMG_OUT = N * PXL               # floats per image output
IMGS_PER_CORE = 2
N_CORES = 8

G = 4                           # y-rows per tile
NT = 36 // G                    # 9 tiles (primary covers rows 0..35)
TPP = 12 // G                   # 3 tiles per phase (dup covers 12 rows)
PRIM_W = 42                     # primary row window [0,42)
DUP_W = 18                      # dup row window [36,54)
WIN = KW * D                    # 448 floats per (j,d) window row
PROW = PRIM_W * WIN             # 18816 floats per primary lane
DROW = DUP_W * WIN              # 8064 floats per dup lane
EXPF = G * PXL                  # 12544 floats per lane per exp buffer

_CACHE = {}


def _build_nc():
    import concourse.bass as bass
    import concourse.mybir as mybir

    nc = bass.Bass(trn_type="TRN2")
    xp = nc.dram_tensor("xp", [96, PROW], mybir.dt.bfloat16,
                        kind="ExternalInput")
    xd = nc.dram_tensor("xd", [3, 32, DROW], mybir.dt.bfloat16,
                        kind="ExternalInput")
    out = nc.dram_tensor("out", [IMGS_PER_CORE, N, K, D], mybir.dt.float32,
                         kind="ExternalOutput")

    with (
        nc.sbuf_tensor("pad7", [128, PROW], mybir.dt.bfloat16) as pad7,
        nc.sbuf_tensor("exp0", [128, EXPF], mybir.dt.float32) as exp0,
        nc.sbuf_tensor("exp1", [128, EXPF], mybir.dt.float32) as exp1,
        nc.sbuf_tensor("exp2", [128, EXPF], mybir.dt.float32) as exp2,
        nc.semaphore("ld") as ld,
        nc.semaphore("rl") as rl,
        nc.semaphore("cpV") as cpV,
        nc.semaphore("cpD") as cpD,
        nc.semaphore("st0") as st0,
        nc.semaphore("st1") as st1,
        nc.semaphore("st2") as st2,
    ):
        exps = (exp0, exp1, exp2)
        sts = (st0, st1, st2)
        # Initial loads on the gpsimd (SWDGE) ring. Dup region is
        # double-buffered at 96*PROW + (ph%2)*DROW (dup lanes only use
        # DROW of their PROW-wide allocation), so phase reloads never
        # stall the ACT pipeline. xp is split so tile 0 starts early:
        # chunk A = window rows [0,18) covers tiles 0-2 (y<=11, +i<=17).
        # xp chunks: rows [0,10) (tile 0), [10,18) (tiles 1-2),
        # [18,42) (tiles 3+); tile t needs window rows <= 4t+9.
        CHUNKS = ((0, 10), (10, 18), (18, PRIM_W))
        nc.gpsimd.dma_start(
            out=bass.AP(pad7, 96 * PROW, [[PROW, 32], [1, DROW]]),
            in_=bass.AP(xd, 0, [[DROW, 32], [1, DROW]]),
        ).then_inc(rl, 16)
        for r0, r1 in CHUNKS:
            nc.gpsimd.dma_start(
                out=bass.AP(pad7, r0 * WIN, [[PROW, 96],
                                             [1, (r1 - r0) * WIN]]),
                in_=bass.AP(xp, r0 * WIN, [[PROW, 96],
                                           [1, (r1 - r0) * WIN]]),
            ).then_inc(ld, 16)
        # Dup phase 1 content preloaded into the second dup slot.
        nc.gpsimd.dma_start(
            out=bass.AP(pad7, 96 * PROW + DROW, [[PROW, 32], [1, DROW]]),
            in_=bass.AP(xd, 32 * DROW, [[DROW, 32], [1, DROW]]),
        ).then_inc(rl, 16)
        nc.vector.wait_ge(ld, 16)
        nc.scalar.wait_ge(rl, 16)

        # Each tile's store DMAs alternate across BOTH rings (a tile's
        # transfers then drain ~2x faster, so the 2-deep exp pipeline
        # doesn't serialize copy->store->copy). Completion is tracked
        # per ring: cum_ring[r][k] = ring-r DMAs through tile k-1.
        def tile_dmalist(t):
            ph = t // TPP
            per_yrow = 3 + (1 if ph == 1 else 0)
            return G * per_yrow

        # Completion sem is per TILE PARITY (all of tile t's DMAs inc
        # sts[t%2]): at wait time only tiles of that parity <= t-2 have
        # been issued, so the count threshold is exactly-issued — no
        # increments can be borrowed from later in-flight tiles.
        cum_tp = {0: [0], 1: [0], 2: [0]}
        ring_of = []            # ring index for each (tile, dma_idx)
        for t in range(NT):
            ph = t // TPP
            n = tile_dmalist(t)
            # Ring 0 = image-0 primaries (one sequential address
            # stream), ring 1 = image-1 primaries + dup stores (mostly
            # sequential): each ring's engine packets walk increasing
            # addresses instead of hopping between regions.
            ndup = 1 if ph != 1 else 2
            rings = ([0, 1] + [1] * ndup) * G
            assert len(rings) == n
            ring_of.append(rings)
            p = t % 3
            for q in (0, 1, 2):
                cum_tp[q].append(cum_tp[q][-1] + (n if q == p else 0))

        rings_nc = (nc.sync, nc.scalar)
        for t in range(NT):
            ph = t // TPP
            buf = exps[t % 3]

            # exp[buf] free once tile t-3's stores completed.
            if t >= 3:
                nc.vector.wait_ge(sts[t % 3], 16 * cum_tp[t % 3][t - 2])
                nc.scalar.wait_ge(sts[t % 3], 16 * cum_tp[t % 3][t - 2])
            if t in (1, 3):
                # chunk 2 (rows<=17) ready for tile 1-2; chunk 3 for 3+
                nc.vector.wait_ge(ld, 32 if t == 1 else 48)
            if t == 0:
                # Reload dup slot 0 with phase-2 content once phase 0's
                # ACT copies are done (queued on gpsimd up-front).
                nc.gpsimd.wait_ge(cpD, TPP)
                nc.gpsimd.dma_start(
                    out=bass.AP(pad7, 96 * PROW, [[PROW, 32], [1, DROW]]),
                    in_=bass.AP(xd, 2 * 32 * DROW, [[DROW, 32], [1, DROW]]),
                ).then_inc(rl, 16)
            if t % TPP == 0 and ph > 0:
                nc.scalar.wait_ge(rl, 16 * (ph + 1))

            # Primary gather (DVE, one instr, 448-elem runs):
            # exp[p, (yrow,i,j,d)] = pad7[p, (Gt+yrow+i, j, d)]
            nc.vector.tensor_copy(
                out=bass.AP(buf, 0,
                            [[EXPF, 96], [PXL, G], [WIN, KH], [1, WIN]]),
                in_=bass.AP(pad7, G * t * WIN,
                            [[PROW, 96], [WIN, G], [WIN, KH], [1, WIN]]),
            ).then_inc(cpV, 1)
            # Dup gather (ACT): rows 36+G*(t%TPP)+yrow, dup slot ph%2.
            nc.scalar.copy(
                out=bass.AP(buf, 96 * EXPF,
                            [[EXPF, 32], [PXL, G], [WIN, KH], [1, WIN]]),
                in_=bass.AP(pad7, 96 * PROW + (ph % 2) * DROW
                            + G * (t % TPP) * WIN,
                            [[PROW, 32], [WIN, G], [WIN, KH], [1, WIN]]),
            ).then_inc(cpD, 1)

            nc.sync.wait_ge(cpV, t + 1)
            nc.sync.wait_ge(cpD, t + 1)
            nc.scalar.wait_ge(cpV, t + 1)
            colbase = 36 + G * (t % TPP)    # dup y-row base
            if ph == 0:       # dup lanes = im0, x 0..31
                dsts = [(0, 0, 32)]
            elif ph == 1:     # im0 x32..47 + im1 x0..15
                dsts = [(0, 32, 16), (1, 0, 16)]
            else:             # im1, x 16..47
                dsts = [(1, 16, 32)]
            di = 0
            for yrow in range(G):
                y = G * t + yrow
                # Primary stores: per image, 48 x 12544B descs with a
                # 48-entry outer dim (engine round-robin follows the
                # outer AP dim; a 1/2-entry outer dim starves engines).
                for im in range(2):
                    r = ring_of[t][di]
                    rings_nc[r].dma_start(
                        out=bass.AP(out, im * IMG_OUT + y * W * PXL,
                                    [[PXL, W], [1, PXL]]),
                        in_=bass.AP(buf, im * 48 * EXPF + yrow * PXL,
                                    [[EXPF, 48], [1, PXL]]),
                    ).then_inc(sts[t % 3], 16)
                    di += 1
                # Dup store(s): y-row colbase+yrow, x per phase subset.
                yd = colbase + yrow
                src_off = 96 * EXPF + yrow * PXL
                for im, x0, nx in dsts:
                    r = ring_of[t][di]
                    rings_nc[r].dma_start(
                        out=bass.AP(out, im * IMG_OUT
                                    + (yd * W + x0) * PXL,
                                    [[PXL, nx], [1, PXL]]),
                        in_=bass.AP(buf, src_off, [[EXPF, nx], [1, PXL]]),
                    ).then_inc(sts[t % 3], 16)
                    src_off += nx * EXPF
                    di += 1

        for eng in (nc.sync, nc.scalar, nc.gpsimd, nc.vector):
            eng.wait_ge(st0, 16 * cum_tp[0][NT])
            eng.wait_ge(st1, 16 * cum_tp[1][NT])
            eng.wait_ge(st2, 16 * cum_tp[2][NT])
    return nc


def _in_maps_from_x(x):
    """Host prep: per-x column windows in (row, j, d) order per lane."""
    x = np.asarray(x, dtype=np.float32)
    b, nh = x.shape[0], x.shape[1]
    img = np.ascontiguousarray(x.reshape(b * nh, H, W, D))
    in_maps = []
    for c in range(N_CORES):
        P = np.zeros((IMGS_PER_CORE, H + 6, W + 6, D), dtype=np.float32)
        P[:, 3:3 + H, 3:3 + W, :] = img[IMGS_PER_CORE * c:
                                        IMGS_PER_CORE * (c + 1)]
        # Bw[im, r, j, x, d] = P[im, r, x+j, d]
        Bw = np.stack([P[:, :, j:j + W, :] for j in range(KH)], axis=2)
        # lane (im,x) content (r, j, d): transpose to (im, x, r, j, d)
        import ml_dtypes
        bf16 = ml_dtypes.bfloat16
        xp = np.ascontiguousarray(
            Bw[:, 0:PRIM_W].transpose(0, 3, 1, 2, 4)).reshape(
            96, PROW).astype(bf16)
        xd = np.ascontiguousarray(
            Bw[:, 36:36 + DUP_W].transpose(0, 3, 1, 2, 4)).reshape(
            96, DROW).astype(bf16)
        in_maps.append({"xp": xp, "xd": xd.reshape(3, 32, DROW)})
    return in_maps


def kernel(x, height=48, width=48):
    from concourse.bass_utils import run_bass_kernel_spmd

    in_maps = _in_maps_from_x(x)
    if "nc" not in _CACHE:
        _CACHE["nc"] = _build_nc()
    res = run_bass_kernel_spmd(_CACHE["nc"], in_maps, core_ids=list(range(N_CORES)))
    y = np.stack([res.results[c]["out"] for c in range(N_CORES)])
    b, nh = np.asarray(x).shape[0], np.asarray(x).shape[1]
    return y.reshape(b, nh, N, K, D).astype(np.float32, copy=False)
